# revision 26
# baseline (speedup 1.0000x reference)
"""Tensor-parallel GQA attention layer (T=2048, dim=4096, 32 q-heads / 8 kv-heads,
D=128, interleaved RoPE, causal) for 8 Trainium2 NeuronCores.

Sharding: TP over heads. Each core owns 4 q-heads + 1 kv-head:
  - w_qkv rows (head-grouped) sharded -> per-core [768, 4096]
  - w_o columns sharded -> per-core [4096, 512]
  - x replicated
Each core computes its partial output [2048, 4096] in bf16; the host sums the
8 partials in fp32 (equivalent to the all-reduce) and casts to bf16.

Device layout (per core) -- everything kept "transposed" so each stage's
output is the next stage's natural PE operand.  One top-level PSUM pool is
shared by both phases (tag rings: "acc" x5 for qkv accumulators, rope
pair-swap AND score tiles; "r" x1; "u" x2 for U and w_o psums) so there is
no pool-release barrier between phases -- the PE flows from the last rope
matmul straight into the first score matmul.

  phase 1 (t-quarters of 512): qkv^T = (w_qkvT tiles).T @ xT tiles.
    x is host-preblocked to [128, 32, T] so a quarter is four DMAs; chunks
    0-1 are double-buffered (prefetched a quarter ahead), chunks 2-3 load
    at quarter start and are consumed last.  Quarter 0 runs i-outer in two
    passes (jt 0-3, then 4-5) so PE consumption tracks the w_i DMA ramp.
    Quarters 1-3 roll jt-by-jt; each jt's rope chain (psum->bf16 copy,
    pair-swap matmul IN PLACE, cos/sin blend) is emitted behind the next
    jt's matmul block so the PE never waits on the DVE.  V blocks come
    from one DMA-transpose per quarter into a [128, 4, 128] tile.
  phase 2+3 fused, j-outer: per 512-col t-chunk j, each head runs
    S^T = KrT.T @ QrT -> exp(scale*s - 2) -> P^T; full s-blocks are written
    as fp8 pairs so the row-sum pass uses one DoubleRow fp8 matmul per pair
    (ones8 stationary); diagonal blocks stay bf16 + triangular mask. U^T
    accumulates V-tiles against the (fp8/bf16) P^T. AO^T = U^T/r.  One
    w_o work unit (4 matmuls + a psum copy) from the previous chunk is
    emitted after every score matmul, so the PE always has fill work while
    the exps drain on the ACT engine.
  The exp bias of -2 rescales P and r equally, cancelling in AO; it keeps
  exp outputs comfortably inside fp8 range.
"""
import numpy as np
import ml_dtypes

T, DIM, H, HKV, D, NCORES = 2048, 4096, 32, 8, 128, 8
HL = H // NCORES            # 4 local q heads
JL = (HL + 2) * D           # 768 local qkv rows
WO_L = HL * D               # 512 local w_o cols
SCALE = float(D) ** -0.5
THETA = 10000.0
NP_BF16 = ml_dtypes.bfloat16
TQ = 512                    # t-quarter width
NQ = T // TQ                # 4 quarters
NST = T // D                # 16 s-blocks

_CACHE = {}


def _build_nc(reps=1):
    from contextlib import ExitStack
    import concourse.bacc as bacc
    import concourse.mybir as mybir
    from concourse.tile import TileContext

    bf = mybir.dt.bfloat16
    f32 = mybir.dt.float32
    f8 = mybir.dt.float8e4
    Exp = mybir.ActivationFunctionType.Exp
    Copy = mybir.ActivationFunctionType.Copy
    DR = mybir.MatmulPerfMode.DoubleRow

    nc = bacc.Bacc("TRN2", target_bir_lowering=False, debug=False,
                   num_devices=NCORES)
    # x pre-blocked on host: xTb[p, i, t] = x.T[i*128 + p, t]
    xb_h = nc.dram_tensor("xTb", [D, 32, T], bf, kind="ExternalInput")
    wq_h = nc.dram_tensor("wqkvT", [DIM, JL], bf, kind="ExternalInput")
    wo_h = nc.dram_tensor("woT", [WO_L, DIM], bf, kind="ExternalInput")
    cos_h = nc.dram_tensor("cosb", [D, T], f32, kind="ExternalInput")
    sin_h = nc.dram_tensor("sinb", [D, T], f32, kind="ExternalInput")
    psw_h = nc.dram_tensor("pswap", [D, D], bf, kind="ExternalInput")
    msk_h = nc.dram_tensor("mask01", [D, D], bf, kind="ExternalInput")
    out_h = nc.dram_tensor("outp", [T, DIM], bf, kind="ExternalOutput")
    xTb, wq, wo = xb_h.ap(), wq_h.ap(), wo_h.ap()
    cosb, sinb, psw, msk, outp = cos_h.ap(), sin_h.ap(), psw_h.ap(), msk_h.ap(), out_h.ap()

    def emit_once(tc, top, S, first):
        if first:
            const = top.enter_context(tc.tile_pool(name="const", bufs=1))
            S["pswap_sb"] = const.tile([D, D], bf, name="pswap_sb")
            S["mask_sb"] = const.tile([D, D], bf, name="mask_sb")
            ones_sb = const.tile([D, D], bf, name="ones_sb")
            ones8_sb = const.tile([D, 2, D], f8, name="ones8_sb")
            neg2_sb = const.tile([D, 1], f32, name="neg2_sb")
            nc.vector.memset(ones_sb[:], 1.0)
            nc.vector.memset(ones8_sb[:], 1.0)
            nc.vector.memset(neg2_sb[:], -2.0)
            S["ones_sb"], S["ones8_sb"], S["neg2_sb"] = \
                ones_sb, ones8_sb, neg2_sb

            persist = top.enter_context(tc.tile_pool(name="persist", bufs=1))
            # Qr^T per local head + Kr^T, one tile per quarter so phase-2
            # reads depend only on the producing quarter's rope (exact deps)
            S["rot_sb"] = [[persist.tile([D, TQ], bf, name=f"rot{jt}_{q}",
                                         tag=f"rot{jt}_{q}")
                            for q in range(NQ)] for jt in range(HL + 1)]
            # V blocked [td, st%4, d], one tile per quarter (+fp8 copy
            # for the DoubleRow U-pass on full s-blocks)
            S["v4_sb"] = [persist.tile([D, 4, D], bf, name=f"v4_{q}",
                                       tag=f"v4_{q}") for q in range(NQ)]
            S["v8_sb"] = [persist.tile([D, 4, D], f8, name=f"v8_{q}",
                                       tag=f"v8_{q}") for q in range(NQ)]
            # AO^T per local head
            S["ao_sb"] = [persist.tile([D, T], bf, name=f"ao{h}",
                                       tag=f"ao{h}") for h in range(HL)]

            # one psum pool shared by both phases AND across bodies:
            # 5 + 1 + 2 = 8 banks
            S["pp"] = top.enter_context(
                tc.tile_pool(name="pp", bufs=1, space="PSUM"))
            S["pt8"] = top.enter_context(tc.tile_pool(name="pt8", bufs=6))
            S["ptd"] = top.enter_context(tc.tile_pool(name="ptd", bufs=4))
            S["rip"] = top.enter_context(tc.tile_pool(name="rip", bufs=1))
            S["wp"] = top.enter_context(tc.tile_pool(name="wp", bufs=1))
            S["xp"] = top.enter_context(tc.tile_pool(name="xp", bufs=1))
            S["rp"] = top.enter_context(tc.tile_pool(name="rp", bufs=1))
            S["sp"] = top.enter_context(tc.tile_pool(name="sp", bufs=1))
            S["tp"] = top.enter_context(tc.tile_pool(name="tp", bufs=1))
            S["wop"] = top.enter_context(tc.tile_pool(name="wop", bufs=1))
            S["obp"] = top.enter_context(tc.tile_pool(name="obp", bufs=2))
        pswap_sb, mask_sb = S["pswap_sb"], S["mask_sb"]
        ones_sb, ones8_sb, neg2_sb = \
            S["ones_sb"], S["ones8_sb"], S["neg2_sb"]
        rot_sb, v4_sb, ao_sb = S["rot_sb"], S["v4_sb"], S["ao_sb"]
        v8_sb = S["v8_sb"]
        pp, pt8, ptd, rip = S["pp"], S["pt8"], S["ptd"], S["rip"]
        wp, xp, rp, sp, tp = S["wp"], S["xp"], S["rp"], S["sp"], S["tp"]

        def v_ap(st):
            return v4_sb[st // 4][:, st % 4, :]

        def acc_tile(name):
            return pp.tile([D, TQ], f32, name=name, tag="acc", bufs=5)

        # ---------------- phase 1: QKV + RoPE + V (t-quarters) ----------------
        if True:

            def x_chunk(q, cc):
                gl = slice(q * TQ, (q + 1) * TQ)
                xt = xp.tile([D, 8, TQ], bf, name=f"xc{cc}", tag=f"xc{cc}",
                             bufs=2 if cc < 2 else 1)
                nc.sync.dma_start(xt[:], xTb[:, cc * 8:(cc + 1) * 8, gl])
                return xt

            def cs_dmas(q):
                gl = slice(q * TQ, (q + 1) * TQ)
                ct = rp.tile([D, TQ], f32, name="cosq", tag="cosq")
                st_ = rp.tile([D, TQ], f32, name="sinq", tag="sinq")
                nc.sync.dma_start(ct[:], cosb[:, gl])
                nc.sync.dma_start(st_[:], sinb[:, gl])
                return ct, st_

            # quarter 0: x chunks interleaved with per-i w DMAs so the PE's
            # i-ordered consumption starts ~2us in and never starves.  The
            # weights stay resident, so only the first body loads them.
            xh = [None] * 4
            if first:
                w_sb = [None] * 32
                S["w_sb"] = w_sb

                def w_dma(i):
                    wt = wp.tile([D, JL], bf, name=f"w{i}", tag=f"w{i}")
                    nc.sync.dma_start(wt[:], wq[i * 128:(i + 1) * 128, :])
                    w_sb[i] = wt

                w_dma(0)
                for cc in range(4):
                    xh[cc] = x_chunk(0, cc)
                    for i in range(cc * 8, cc * 8 + 8):
                        if w_sb[i] is None:
                            w_dma(i)
                cos_q, sin_q = cs_dmas(0)
                nc.sync.dma_start(pswap_sb[:], psw[:, :])
                nc.sync.dma_start(mask_sb[:], msk[:, :])
            else:
                w_sb = S["w_sb"]
                for cc in range(4):
                    xh[cc] = x_chunk(0, cc)
                cos_q, sin_q = cs_dmas(0)

            def rope_chain(jt, ps_jt, cos_q, sin_q, q):
                qt = sp.tile([D, TQ], bf, name=f"qb{jt}", tag=f"qb{jt}")
                nc.vector.tensor_copy(qt[:], ps_jt[:])
                nc.tensor.matmul(ps_jt[:], pswap_sb[:], qt[:],
                                 start=True, stop=True)
                t1 = tp.tile([D, TQ], f32, name="t1", tag="t1")
                t2t = tp.tile([D, TQ], f32, name="t2t", tag="t2t")
                nc.vector.tensor_mul(t1[:], qt[:], cos_q[:])
                nc.vector.tensor_mul(t2t[:], ps_jt[:], sin_q[:])
                nc.vector.tensor_add(rot_sb[jt][q][:], t1[:], t2t[:])

            def v_chain(q, ps_v):
                vb = sp.tile([D, TQ], bf, name="vb", tag="vb")
                nc.vector.tensor_copy(vb[:], ps_v[:])
                nc.sync.dma_start_transpose(v4_sb[q][:], vb[:])
                nc.vector.tensor_copy(v8_sb[q][:], v4_sb[q][:])

            for q in range(NQ):
                gl = slice(q * TQ, (q + 1) * TQ)
                if q == 0:
                    # two i-outer passes (4 jts, then 2) within the 5-slot ring
                    psA = [acc_tile(f"qkv_ps{jt}") for jt in range(4)]
                    for i in range(32):
                        for jt in range(4):
                            nc.tensor.matmul(
                                psA[jt][:], w_sb[i][:, jt * 128:(jt + 1) * 128],
                                xh[i // 8][:, i % 8, :],
                                start=(i == 0), stop=(i == 31))
                    for jt in range(4):
                        rope_chain(jt, psA[jt], cos_q, sin_q, 0)
                    psB = [acc_tile(f"qkv_ps{jt}") for jt in (4, 5)]
                    for pos, i in enumerate(
                            list(range(16, 32)) + list(range(16))):
                        for k, jt in enumerate((4, 5)):
                            nc.tensor.matmul(
                                psB[k][:], w_sb[i][:, jt * 128:(jt + 1) * 128],
                                xh[i // 8][:, i % 8, :],
                                start=(pos == 0), stop=(pos == 31))
                    rope_chain(4, psB[0], cos_q, sin_q, 0)
                    v_chain(0, psB[1])
                else:
                    # rolling jt: each jt's rope chain is emitted behind the
                    # next jt's matmul block
                    prev = None
                    for jt in range(6):
                        ps_jt = acc_tile(f"qkv_ps{jt}")
                        iorder = range(32) if jt < 4 else \
                            list(range(16, 32)) + list(range(16))
                        for pos, i in enumerate(iorder):
                            nc.tensor.matmul(
                                ps_jt[:], w_sb[i][:, jt * 128:(jt + 1) * 128],
                                xh[i // 8][:, i % 8, :],
                                start=(pos == 0), stop=(pos == 31))
                        if jt == 2 and q + 1 < NQ:
                            # prefetch next quarter's double-buffered chunks
                            nxt_xh01 = [x_chunk(q + 1, 0), x_chunk(q + 1, 1)]
                        if prev is not None:
                            rope_chain(prev[0], prev[1], cos_q, sin_q, q)
                        prev = (jt, ps_jt)
                    v_chain(q, prev[1])
                if q + 1 < NQ:
                    if q == 0:
                        nxt_xh01 = [x_chunk(1, 0), x_chunk(1, 1)]
                    xh[0], xh[1] = nxt_xh01
                    xh[2], xh[3] = x_chunk(q + 1, 2), x_chunk(q + 1, 3)
                    cos_q, sin_q = cs_dmas(q + 1)

        # w_o prefetch: first body only (resident afterwards)
        if first:
            wo_sb = []
            for jc in range(HL):
                wt = S["wop"].tile([D, DIM], bf, name=f"wo{jc}", tag=f"wo{jc}")
                nc.sync.dma_start(wt[:], wo[jc * 128:(jc + 1) * 128, :])
                wo_sb.append(wt)
            S["wo_sb"] = wo_sb
        wo_sb = S["wo_sb"]
        obp = S["obp"]

        # ---------------- phase 2+3 fused: attention + output proj ----------------
        if True:

            # w_o work queue: units of (tt, mb) -- 4 matmuls + 1 copy each.
            # One unit is emitted after every score matmul, so the PE always
            # has fill work while this head's exps drain on the ACT engine.
            wo_state = {"units": [], "ob": None}

            def emit_wo_unit():
                if not wo_state["units"]:
                    return
                tt, mb = wo_state["units"].pop(0)
                if mb % 4 == 0:
                    wo_state["ob"] = obp.tile([D, 2048], bf, name="ob",
                                              tag="ob")
                ob = wo_state["ob"]
                po = pp.tile([D, 512], f32, name="po", tag="u", bufs=2)
                m0 = mb * 512
                for jc in range(HL):
                    nc.tensor.matmul(
                        po[:], ao_sb[jc][:, tt * 128:(tt + 1) * 128],
                        wo_sb[jc][:, m0:m0 + 512],
                        start=(jc == 0), stop=(jc == HL - 1))
                osl = slice(m0 % 2048, m0 % 2048 + 512)
                if mb % 2 == 0:
                    nc.vector.tensor_copy(ob[:, osl], po[:])
                else:
                    nc.scalar.activation(ob[:, osl], po[:], Copy)
                if mb % 4 == 3:
                    half = mb // 4
                    nc.sync.dma_start(
                        outp[tt * 128:(tt + 1) * 128,
                             half * 2048:(half + 1) * 2048], ob[:])

            for j in range(4):              # t-chunks of 512
                jsl = slice(j * 512, (j + 1) * 512)
                n_st = 4 * j + 4
                for h in range(HL):
                    ksc = 0
                    # full s-blocks as fp8 pairs
                    pairs = []
                    for b in range(2 * j):
                        p2 = pt8.tile([D, 2, 512], f8, name="p2", tag="p2")
                        for ko in range(2):
                            st = 2 * b + ko
                            sc = acc_tile("sc")
                            nc.tensor.matmul(
                                sc[:],
                                rot_sb[HL][st // 4][:, (st % 4) * 128:
                                                    (st % 4 + 1) * 128],
                                rot_sb[h][j][:], start=True, stop=True)
                            nc.scalar.activation(p2[:, ko, :], sc[:], Exp,
                                                 bias=neg2_sb[:], scale=SCALE)
                            ksc += 1
                            if ksc > n_st - 8:
                                emit_wo_unit()
                        pairs.append(p2)
                    # diagonal s-blocks: bf16 + triangular mask
                    diags = []
                    for d4 in range(4):
                        st = 4 * j + d4
                        t_off = d4 * 128
                        sc = acc_tile("sc")
                        nc.tensor.matmul(
                            sc[:, t_off:],
                            rot_sb[HL][st // 4][:, (st % 4) * 128:
                                                (st % 4 + 1) * 128],
                            rot_sb[h][j][:, t_off:],
                            start=True, stop=True)
                        pd = ptd.tile([D, 512], bf, name="pd", tag="pd")
                        nc.scalar.activation(pd[:, t_off:], sc[:, t_off:], Exp,
                                             bias=neg2_sb[:], scale=SCALE)
                        nc.vector.tensor_mul(pd[:, t_off:t_off + 128],
                                             pd[:, t_off:t_off + 128],
                                             mask_sb[:])
                        diags.append(pd)
                        ksc += 1
                        if ksc > n_st - 8:
                            emit_wo_unit()
                    # drain this head's share of w_o units
                    while len(wo_state["units"]) > (HL - 1 - h) * 8:
                        emit_wo_unit()
                    # r: DoubleRow fp8 per pair + bf16 diagonals
                    r_ps = pp.tile([D, 512], f32, name="r_ps", tag="r", bufs=1)
                    n_r = 2 * j + 4
                    idx = 0
                    for b in range(2 * j):
                        nc.tensor.matmul(r_ps[:], ones8_sb[:, :, :],
                                         pairs[b][:, :, :],
                                         start=(idx == 0), stop=(idx == n_r - 1),
                                         perf_mode=DR)
                        idx += 1
                    for d4 in range(4):
                        t_off = d4 * 128
                        nc.tensor.matmul(r_ps[:, t_off:], ones_sb[:],
                                         diags[d4][:, t_off:],
                                         start=(idx == 0), stop=(idx == n_r - 1))
                        idx += 1
                    # U: DoubleRow fp8 V-pairs on full blocks + bf16 diags
                    u_ps = pp.tile([D, 512], f32, name="u_ps", tag="u", bufs=2)
                    n_u = 2 * j + 4
                    idx = 0
                    for b in range(2 * j):
                        v2 = v8_sb[b // 2][:, 2 * (b % 2):2 * (b % 2) + 2, :]
                        nc.tensor.matmul(u_ps[:], v2, pairs[b][:, :, :],
                                         start=(idx == 0), stop=(idx == n_u - 1),
                                         perf_mode=DR)
                        idx += 1
                    for d4 in range(4):
                        st = 4 * j + d4
                        t_off = d4 * 128
                        nc.tensor.matmul(u_ps[:, t_off:], v_ap(st),
                                         diags[d4][:, t_off:],
                                         start=(idx == 0), stop=(idx == n_u - 1))
                        idx += 1
                    r_inv = rip.tile([D, 512], f32, name="r_inv", tag="ri")
                    nc.vector.reciprocal(r_inv[:], r_ps[:])
                    nc.vector.tensor_mul(ao_sb[h][:, jsl], u_ps[:], r_inv[:])

                # queue this chunk's w_o units; they are drained inside the
                # next chunk's head loop (or below for the last chunk)
                for tt in range(4 * j, 4 * j + 4):
                    wo_state["units"].extend((tt, mb) for mb in range(8))
            while wo_state["units"]:
                emit_wo_unit()

    from contextlib import ExitStack as _ES
    with TileContext(nc) as tc:
        with _ES() as top:
            S = {}
            for _rep in range(reps):
                emit_once(tc, top, S, first=(_rep == 0))

    nc.compile()
    return nc


def get_nc(reps=1):
    key = ("nc", reps)
    if key not in _CACHE:
        _CACHE[key] = _build_nc(reps)
    return _CACHE[key]


def host_prep(x, w_qkv, w_o):
    """Returns per-core input maps (numpy)."""
    x = np.asarray(x)
    w_qkv = np.asarray(w_qkv)
    w_o = np.asarray(w_o)
    xT = np.ascontiguousarray(x.T)                       # [DIM, T]
    xTb = np.ascontiguousarray(
        xT.reshape(32, D, T).transpose(1, 0, 2))         # [128, 32, T]
    inv_freq = 1.0 / (THETA ** (np.arange(0, D, 2, dtype=np.float64) / D))
    ang = np.arange(T, dtype=np.float64)[:, None] * inv_freq[None, :]
    cosb = np.empty((D, T), np.float32)
    sinb = np.empty((D, T), np.float32)
    cosb[0::2] = np.cos(ang).T
    cosb[1::2] = np.cos(ang).T
    sinb[0::2] = -np.sin(ang).T
    sinb[1::2] = np.sin(ang).T
    pswap = np.zeros((D, D), NP_BF16)
    for d in range(D):
        pswap[d, d ^ 1] = 1
    mask01 = np.triu(np.ones((128, 128), np.float32)).astype(NP_BF16)
    in_maps = []
    for c in range(NCORES):
        wq_rows = w_qkv[c * HL * D:(c + 1) * HL * D]
        wk_rows = w_qkv[H * D + c * D: H * D + (c + 1) * D]
        wv_rows = w_qkv[(H + HKV) * D + c * D:(H + HKV) * D + (c + 1) * D]
        w_c = np.concatenate([wq_rows, wk_rows, wv_rows], axis=0)
        in_maps.append({
            "xTb": xTb,
            "wqkvT": np.ascontiguousarray(w_c.T),
            "woT": np.ascontiguousarray(w_o[:, c * WO_L:(c + 1) * WO_L].T),
            "cosb": cosb, "sinb": sinb, "pswap": pswap, "mask01": mask01,
        })
    return in_maps


def kernel(x, w_qkv, w_o):
    from concourse.bass_utils import run_bass_kernel_spmd
    nc = get_nc()
    in_maps = host_prep(x, w_qkv, w_o)
    res = run_bass_kernel_spmd(nc, in_maps, list(range(NCORES)))
    acc = np.zeros((T, DIM), np.float32)
    for c in range(NCORES):
        acc += res.results[c]["outp"].astype(np.float32)
    return acc.astype(NP_BF16)
